# revision 1
# baseline (speedup 1.0000x reference)
"""Trainium2 Bass kernel for DevConv-style GNN message passing.

Reference computation:
    rel_t = (x[row] - x[col]) @ W_theta.T          # [E, 128]
    aggr  = segment_max(rel_t, row, N)             # [N, 128], empty -> 0
    out   = aggr @ W_phi.T                         # [N, 128]

Key reformulation: with y = x @ W_theta.T, within a segment (fixed row d)
    max_e (y[d] - y[col_e]) = y[d] - min_e y[col_e]     (per channel)
so the per-edge matmul disappears and only ONE gather per edge (y[col]) is
needed, followed by a segmented min.

Distribution: nodes are assigned to the 8 cores by degree-rank striping
(rank r -> core r % 8). Each core:
  Phase A: computes y = x @ W_theta.T for ALL nodes (bf16) into an HBM
           table that is split into 4 chunks of <=32767 rows (+1 sentinel
           row of +3e38 per chunk) because dma_gather indices are int16.
  Phase B: for each 128-node tile and each chunk, gathers y[col] rows into
           a padded [128 nodes x B slots] SBUF rect via dma_gather
           (pad slots point at the chunk sentinel), then pairwise-min folds
           the slots and merges chunks -> m[d] = min_e y[col_e].
  Phase C: aggr = y_own - m  (y_own computed on-chip from x_own),
           out_tile = aggr @ W_phi.T via PE transpose + matmul.
Host un-permutes the concatenated core outputs and zeroes empty nodes.
"""
import sys
import os

sys.path.insert(0, "/opt/trn_rl_repo")

from contextlib import ExitStack
from dataclasses import dataclass

import numpy as np
import ml_dtypes

import concourse.bass as bass
import concourse.tile as tile
from concourse import bacc, mybir
from concourse.masks import make_identity

import time

import jax
from jax.sharding import Mesh, PartitionSpec
from jax.experimental.shard_map import shard_map

from concourse.bass2jax import (
    _bass_exec_p, install_neuronx_cc_hook, partition_id_tensor)


class BassRunner:
    """Keeps a jitted PJRT executable for a Bass program so it can be run
    repeatedly on device-resident inputs (for wall-clock timing)."""

    def __init__(self, nc, n_cores: int):
        install_neuronx_cc_hook()
        self.nc = nc
        self.n_cores = n_cores
        partition_name = nc.partition_id_tensor.name if nc.partition_id_tensor else None
        in_names, out_names, out_avals = [], [], []
        for alloc in nc.m.functions[0].allocations:
            if not isinstance(alloc, mybir.MemoryLocationSet):
                continue
            name = alloc.memorylocations[0].name
            if alloc.kind == "ExternalInput":
                if name != partition_name:
                    in_names.append(name)
            elif alloc.kind == "ExternalOutput":
                out_names.append(name)
                out_avals.append(jax.core.ShapedArray(
                    tuple(alloc.tensor_shape), mybir.dt.np(alloc.dtype)))
        self.in_names, self.out_names, self.out_avals = in_names, out_names, out_avals
        self.n_params = len(in_names)
        all_in_names = list(in_names) + list(out_names)
        if partition_name is not None:
            all_in_names.append(partition_name)

        def _body(*args):
            operands = list(args)
            if partition_name is not None:
                operands.append(partition_id_tensor())
            outs = _bass_exec_p.bind(
                *operands,
                out_avals=tuple(out_avals),
                in_names=tuple(all_in_names),
                out_names=tuple(out_names),
                lowering_input_output_aliases=(),
                sim_require_finite=True,
                sim_require_nnan=True,
                nc=nc,
            )
            return tuple(outs)

        devices = jax.devices()[:n_cores]
        self.mesh = Mesh(np.asarray(devices), ("core",))
        n_outs = len(out_names)
        in_specs = (PartitionSpec("core"),) * (self.n_params + n_outs)
        out_specs = (PartitionSpec("core"),) * n_outs
        self.fn = jax.jit(
            shard_map(_body, mesh=self.mesh, in_specs=in_specs,
                      out_specs=out_specs, check_rep=False),
            keep_unused=True,
        )
        self._dev_args = None

    def prepare(self, in_maps):
        assert len(in_maps) == self.n_cores
        concat_in = [
            np.concatenate([np.asarray(in_maps[c][n]) for c in range(self.n_cores)],
                           axis=0)
            for n in self.in_names
        ]
        concat_zeros = [
            np.zeros((self.n_cores * a.shape[0], *a.shape[1:]), a.dtype)
            for a in self.out_avals
        ]
        sharding = jax.sharding.NamedSharding(self.mesh, PartitionSpec("core"))
        self._dev_args = [jax.device_put(v, sharding) for v in concat_in + concat_zeros]
        return self

    def run(self):
        outs = self.fn(*self._dev_args)
        jax.block_until_ready(outs)
        return outs

    def results(self, outs):
        return [
            {n: np.asarray(outs[i]).reshape(self.n_cores, *self.out_avals[i].shape)[c]
             for i, n in enumerate(self.out_names)}
            for c in range(self.n_cores)
        ]

    def time_ns(self, iters=5, warmup=2):
        for _ in range(warmup):
            self.run()
        ts = []
        for _ in range(iters):
            t0 = time.perf_counter()
            self.run()
            ts.append((time.perf_counter() - t0) * 1e9)
        return min(ts)


CH = 128
F32 = mybir.dt.float32
BF16 = mybir.dt.bfloat16
I16 = mybir.dt.int16
SENT_VAL = 3.0e38
IDX_PER_INST = 1024          # dma_gather crashes >= 2048 idx/instruction
BLK_PER_INST = IDX_PER_INST // 128


@dataclass(frozen=True)
class Cfg:
    N: int = 100_000
    E: int = 3_200_000
    n_cores: int = 8
    chunk_real: int = 32_767     # rows addressable by int16 (sentinel at chunk_real)

    @property
    def n_chunks(self):
        return (self.N + self.chunk_real - 1) // self.chunk_real

    @property
    def chunk_stride(self):
        return self.chunk_real + 1

    @property
    def npc(self):
        assert self.N % self.n_cores == 0
        return self.N // self.n_cores

    @property
    def tiles(self):
        return (self.npc + 127) // 128


def _wrap16(seg: np.ndarray) -> np.ndarray:
    """Per-instruction idx wrap: flat [n] -> [128, n//16]; idx i at
    (partition i%16, col i//16), replicated across the 8 gpsimd groups."""
    n = seg.shape[-1]
    w = seg.reshape(*seg.shape[:-1], n // 16, 16)
    w = np.swapaxes(w, -1, -2)                       # [..., 16, n//16]
    return np.tile(w, (1,) * (seg.ndim - 1) + (8, 1))


def prep(x, edge_index, cfg: Cfg):
    """Host-side data prep. Returns (plan, per-core inputs, unpermute info)."""
    N, E, NC = cfg.N, cfg.E, cfg.n_cores
    CR, NK, T = cfg.chunk_real, cfg.n_chunks, cfg.tiles
    row = np.asarray(edge_index[0], dtype=np.int64)
    col = np.asarray(edge_index[1], dtype=np.int64)

    deg = np.bincount(row, minlength=N)
    order = np.argsort(-deg, kind="stable")          # node ids by desc degree
    core_of = np.empty(N, np.int64)
    pos_of = np.empty(N, np.int64)
    r = np.arange(N)
    core_of[order] = r % NC
    pos_of[order] = r // NC

    ec = core_of[row]
    ep = pos_of[row]
    ek = col // CR
    elocal = (col - ek * CR).astype(np.int16)
    et = ep // 128
    ed = ep % 128

    # per-(core,tile,chunk,node) counts and within-group slot index j
    key = ((ec * T + et) * NK + ek) * 128 + ed
    o = np.argsort(key, kind="stable")
    ks = key[o]
    first = np.r_[True, ks[1:] != ks[:-1]]
    run_id = np.cumsum(first) - 1
    run_start = np.flatnonzero(first)
    j = np.arange(E) - run_start[run_id]

    cnt = np.bincount(key, minlength=NC * T * NK * 128).reshape(NC, T, NK, 128)
    B = cnt.max(axis=(0, 3)).astype(np.int64)        # [T, NK] shared structure

    Bf = B.reshape(-1)
    off = np.concatenate([[0], np.cumsum(Bf * 128)])  # slot offset per (t,k)
    total_slots = int(off[-1])

    idx_all = np.full((NC, total_slots), CR, np.int16)   # sentinel local idx
    tk = et[o] * NK + ek[o]
    pos_in = off[tk] + j * 128 + ed[o]
    idx_all[ec[o], pos_in] = elocal[o]

    # split into gather instructions and build wrapped idx input
    insts = []           # (t, k, g0blk, nblk, col_off)
    tile_cols = []       # per tile: (col_start, col_end)
    wsegs = []
    col_off = 0
    for t in range(T):
        t_start = col_off
        for k in range(NK):
            btk = int(B[t, k])
            base = int(off[t * NK + k])
            for g0 in range(0, btk, BLK_PER_INST):
                nb = min(BLK_PER_INST, btk - g0)
                n_i = nb * 128
                seg = idx_all[:, base + g0 * 128: base + g0 * 128 + n_i]
                wsegs.append(_wrap16(seg))
                insts.append((t, k, g0, nb, col_off))
                col_off += n_i // 16
        tile_cols.append((t_start, col_off))
    idxw = np.concatenate(wsegs, axis=2) if wsegs else np.zeros((NC, 128, 0), np.int16)
    W_total = idxw.shape[2]

    # per-core x_own in pos order, padded to T*128 rows
    own_nodes = np.empty((NC, cfg.npc), np.int64)
    own_nodes[core_of[order], pos_of[order]] = order  # own_nodes[c, p] = node id
    x_np = np.asarray(x, dtype=np.float32)
    x_own = np.zeros((NC, T * 128, CH), np.float32)
    x_own[:, : cfg.npc] = x_np[own_nodes]

    plan = dict(cfg=cfg, B=B, insts=insts, tile_cols=tile_cols, W_total=W_total)
    return plan, idxw, x_own, own_nodes, deg


def build_program(plan, reps=1, phases="abc"):
    cfg: Cfg = plan["cfg"]
    N, NK, CR, T = cfg.N, cfg.n_chunks, cfg.chunk_real, cfg.tiles
    CS = cfg.chunk_stride
    B, insts, tile_cols, W_total = (
        plan["B"], plan["insts"], plan["tile_cols"], plan["W_total"])

    nc = bacc.Bacc(None, target_bir_lowering=False, num_swdge_queues=4)
    x_full = nc.declare_dram_parameter("x_full", [N, CH], F32, isOutput=False)
    x_own = nc.declare_dram_parameter("x_own", [T * 128, CH], F32, isOutput=False)
    wth = nc.declare_dram_parameter("w_theta_t", [CH, CH], F32, isOutput=False)
    wph = nc.declare_dram_parameter("w_phi_t", [CH, CH], F32, isOutput=False)
    idxw = nc.declare_dram_parameter("idxw", [128, max(W_total, 16)], I16, isOutput=False)
    out = nc.declare_dram_parameter("out", [T * 128, CH], F32, isOutput=True)

    qc = [0]  # gather queue rotation

    with tile.TileContext(nc) as tc:
        with ExitStack() as ctx:
            consts = ctx.enter_context(tc.tile_pool(name="consts", bufs=1))
            dram = ctx.enter_context(tc.tile_pool(name="dram", bufs=1, space="DRAM"))
            ax = ctx.enter_context(tc.tile_pool(name="ax", bufs=2))
            axT = ctx.enter_context(tc.tile_pool(name="axT", bufs=2))
            ay = ctx.enter_context(tc.tile_pool(name="ay", bufs=2))
            ps_t = ctx.enter_context(tc.tile_pool(name="ps_t", bufs=2, space="PSUM"))
            ps_y = ctx.enter_context(tc.tile_pool(name="ps_y", bufs=2, space="PSUM"))
            ps_c = ctx.enter_context(tc.tile_pool(name="ps_c", bufs=2, space="PSUM"))
            gidx = ctx.enter_context(tc.tile_pool(name="gidx", bufs=3))
            gdst = ctx.enter_context(tc.tile_pool(name="gdst", bufs=2))
            fold = ctx.enter_context(tc.tile_pool(name="fold", bufs=2))
            fin = ctx.enter_context(tc.tile_pool(name="fin", bufs=2))

            y_aug = dram.tile([NK * CS, CH], BF16)

            ident = consts.tile([128, 128], F32)
            make_identity(nc, ident[:])
            wth_sb = consts.tile([CH, CH], F32)
            nc.sync.dma_start(out=wth_sb[:], in_=wth[:])
            wph_sb = consts.tile([CH, CH], F32)
            nc.sync.dma_start(out=wph_sb[:], in_=wph[:])
            y_own_sb = consts.tile([128, T * 128], F32)
            probe = consts.tile([128, CH], BF16)
            nc.gpsimd.memset(probe[:], 0.0)
            cst128 = consts.tile([128, CH], BF16)
            nc.gpsimd.memset(cst128[:], 1.0)
            sent = consts.tile([1, CH], BF16)
            nc.gpsimd.memset(sent[:], SENT_VAL)
            for k in range(NK):
                nc.sync.dma_start(out=y_aug[k * CS + CR: k * CS + CR + 1, :], in_=sent[:])

            A_MODE = os.environ.get("A_MODE", "full")
            PS_BUFS = int(os.environ.get("PS_BUFS", "2"))
            # ---------------- Phase A: y_aug = (x @ W_theta.T).bf16 ----------
            def emit_group(src, n0, gn, dst):
                """Process rows [n0, n0+gn) of src -> y into dst.
                dst = ("aug",) writes y_aug rows (with chunk-boundary split),
                dst = ("own",) writes y_own_sb cols."""
                nt = (gn + 127) // 128
                xg = ax.tile([128, nt * 128], F32, tag="xg",
                             bufs=int(os.environ.get("XG_BUFS", "2")))
                xg3 = xg[:].rearrange("p (i c) -> p i c", c=CH)
                load_eng = nc.gpsimd if A_MODE == "dma3" else nc.sync
                if gn % 128 == 0:
                    load_eng.dma_start(
                        out=xg3[:, :nt, :],
                        in_=src[n0: n0 + gn, :].rearrange("(i p) c -> p i c", p=128))
                else:
                    for i in range(nt):
                        rn = min(128, gn - i * 128)
                        nc.sync.dma_start(
                            out=xg3[:rn, i, :],
                            in_=src[n0 + i * 128: n0 + i * 128 + rn, :])
                if A_MODE.startswith("dma") and dst == "aug":
                    # dma : load->store dep, both on sync
                    # dma2: stores only dep-free (loads still emitted)
                    # dma3: load on gpsimd, stores dep on load, on sync
                    # dma5: loads only (no stores)
                    if gn % 128 == 0:
                        for i in range(nt):
                            r0 = n0 + i * 128
                            kb = r0 // CR
                            if A_MODE == "dma5":
                                continue
                            src_ap = (cst128[:] if A_MODE == "dma2"
                                      else xg3[:, i, :CH // 2].bitcast(BF16))
                            nc.sync.dma_start(
                                out=y_aug[r0 + kb: r0 + kb + 128, :], in_=src_ap)
                    return
                pt = ps_t.tile([128, nt * 128], F32, tag="pt", bufs=PS_BUFS)
                for i in range(nt):
                    rn = min(128, gn - i * 128)
                    nc.tensor.transpose(
                        out=pt[:, i * 128: i * 128 + rn],
                        in_=xg3[:rn, i, :],
                        identity=ident[:rn, :rn])
                xT = axT.tile([128, nt * 128], F32, tag="xT")
                nc.vector.tensor_copy(out=xT[:, : nt * 128], in_=pt[:, : nt * 128])
                if A_MODE == "nomm" and dst == "aug":
                    for i in range(nt):
                        r0 = n0 + i * 128
                        kb = r0 // CR
                        nc.sync.dma_start(
                            out=y_aug[r0 + kb: r0 + kb + 128, :],
                            in_=xT[:, i * 128: i * 128 + 128][:, :CH // 2].bitcast(BF16))
                    return
                py = ps_y.tile([128, nt * 128], F32, tag="py", bufs=PS_BUFS)
                for i in range(nt):
                    rn = min(128, gn - i * 128)
                    nc.tensor.matmul(
                        out=py[:rn, i * 128: (i + 1) * 128],
                        lhsT=xT[:, i * 128: i * 128 + rn],
                        rhs=wth_sb[:],
                        start=True, stop=True)
                if dst == "own":
                    nc.scalar.copy(
                        out=y_own_sb[:, n0: n0 + nt * 128], in_=py[:, : nt * 128])
                    return
                yg = ay.tile([128, nt * 128], BF16, tag="yg")
                copy2 = nc.vector.tensor_copy if A_MODE == "dvecopy" else nc.scalar.copy
                if gn % 128 == 0:
                    copy2(out=yg[:, : gn], in_=py[:, : gn])
                else:
                    for i in range(nt):
                        rn = min(128, gn - i * 128)
                        copy2(
                            out=yg[:rn, i * 128: (i + 1) * 128],
                            in_=py[:rn, i * 128: (i + 1) * 128])
                yg3 = yg[:].rearrange("p (i c) -> p i c", c=CH)
                # write y rows n -> aug rows n + n // CR, splitting at tile level
                for i in range(nt):
                    r0 = n0 + i * 128
                    rn = min(128, gn - i * 128)
                    kb = r0 // CR
                    ke = (r0 + rn - 1) // CR
                    if kb == ke:
                        nc.sync.dma_start(
                            out=y_aug[r0 + kb: r0 + kb + rn, :], in_=yg3[:rn, i, :])
                    else:
                        split = (kb + 1) * CR - r0       # rows before boundary
                        nc.sync.dma_start(
                            out=y_aug[r0 + kb: r0 + kb + split, :],
                            in_=yg3[:split, i, :])
                        nc.sync.dma_start(
                            out=y_aug[r0 + split + ke: r0 + ke + rn, :],
                            in_=yg3[split:rn, i, :])

            by_tile = {}
            for (t, k, g0, nb, coff) in insts:
                by_tile.setdefault(t, []).append((k, g0, nb, coff))

            for _rep in range(reps):
              for n0 in range(0, N, 512):
                emit_group(x_full, n0, min(512, N - n0), "aug")
              for n0 in range(0, T * 128, 512):
                emit_group(x_own, n0, min(512, T * 128 - n0), "own")

              # ---------------- Phase B + C per tile ---------------------------
              for t in range(T):
                 c0, c1 = tile_cols[t]
                 it = gidx.tile([128, max(c1 - c0, 16)], I16, tag="it")
                 if c1 > c0:
                     nc.sync.dma_start(out=it[:, : c1 - c0], in_=idxw[:, c0:c1])
                 dks = {}
                 for k in range(NK):
                     btk = int(B[t, k])
                     if btk == 0:
                         continue
                     dks[k] = gdst.tile([128, btk * CH], BF16, tag=f"g{k}", name=f"dk{k}")
                 for (k, g0, nb, coff) in by_tile.get(t, []):
                     dk3 = dks[k][:].rearrange("p (b c) -> p b c", c=CH)
                     n_i = nb * 128
                     nc.gpsimd.dma_gather(
                         out_ap=dk3[:, g0: g0 + nb, :],
                         in_ap=y_aug[k * CS: (k + 1) * CS, :],
                         idxs_ap=it[:, coff - c0: coff - c0 + n_i // 16],
                         num_idxs=n_i,
                         num_idxs_reg=n_i,
                         elem_size=CH,
                         queue_num=qc[0] % 4,
                     )
                     qc[0] += 1
                 # fold each chunk's rect down to one [128, CH] min
                 mks = []
                 for k in range(NK):
                     if k not in dks:
                         continue
                     cur = dks[k]
                     nb = int(B[t, k])
                     while nb > 1:
                         half = (nb + 1) // 2
                         nxt = fold.tile([128, half * CH], BF16, tag=f"f{k}", bufs=3)
                         nc.vector.tensor_tensor(
                             out=nxt[:, : half * CH],
                             in0=cur[:, : half * CH],
                             in1=cur[:, (nb - half) * CH: nb * CH],
                             op=mybir.AluOpType.min)
                         cur, nb = nxt, half
                     mks.append(cur)
                 m = fin.tile([128, CH], F32, tag="m")
                 if len(mks) == 0:
                     nc.gpsimd.memset(m[:], SENT_VAL)
                 elif len(mks) == 1:
                     nc.vector.tensor_copy(out=m[:], in_=mks[0][:, :CH])
                 else:
                     # sequential accumulate with alternating tags (max 2 live)
                     acc = mks[0]
                     for i in range(1, len(mks) - 1):
                         mm = fold.tile([128, CH], BF16, tag=f"mrg{i % 2}")
                         nc.vector.tensor_tensor(
                             out=mm[:], in0=acc[:, :CH], in1=mks[i][:, :CH],
                             op=mybir.AluOpType.min)
                         acc = mm
                     nc.vector.tensor_tensor(
                         out=m[:], in0=acc[:, :CH], in1=mks[-1][:, :CH],
                         op=mybir.AluOpType.min)
                 # aggr = y_own - m ; out_tile = aggr @ W_phi.T
                 aggr = fin.tile([128, CH], F32, tag="aggr")
                 nc.vector.tensor_sub(
                     out=aggr[:], in0=y_own_sb[:, t * 128: (t + 1) * 128], in1=m[:])
                 ptr = ps_c.tile([128, CH], F32, tag="ctr")
                 nc.tensor.transpose(out=ptr[:], in_=aggr[:], identity=ident[:])
                 aggrT = fin.tile([128, CH], F32, tag="aggrT")
                 nc.vector.tensor_copy(out=aggrT[:], in_=ptr[:])
                 po = ps_c.tile([128, CH], F32, tag="cmm")
                 nc.tensor.matmul(out=po[:], lhsT=aggrT[:], rhs=wph_sb[:],
                                  start=True, stop=True)
                 osb = fin.tile([128, CH], F32, tag="osb")
                 nc.scalar.copy(out=osb[:], in_=po[:])
                 nc.sync.dma_start(out=out[t * 128: (t + 1) * 128, :], in_=osb[:])

            if phases != "abc":
                fillz = consts.tile([128, CH], F32)
                nc.vector.tensor_copy(out=fillz[:], in_=probe[:])
                for t in range(T):
                    nc.sync.dma_start(out=out[t * 128: (t + 1) * 128, :], in_=fillz[:])
    nc.compile()
    return nc


_CACHE = {}


def _get_runner_and_plan(x, edge_index, cfg: Cfg, reps=1, phases="abc"):
    plan, idxw, x_own, own_nodes, deg = prep(x, edge_index, cfg)
    skey = (cfg, reps, phases, tuple(plan["B"].reshape(-1).tolist()))
    if skey not in _CACHE:
        nc = build_program(plan, reps=reps, phases=phases)
        _CACHE[skey] = BassRunner(nc, cfg.n_cores)
    return _CACHE[skey], plan, idxw, x_own, own_nodes, deg


def run_cfg(x, edge_index, W_theta, W_phi, cfg: Cfg, time_iters=0, reps=1, phases="abc"):
    runner, plan, idxw, x_own, own_nodes, deg = _get_runner_and_plan(x, edge_index, cfg, reps=reps, phases=phases)
    wtt = np.ascontiguousarray(np.asarray(W_theta, np.float32).T)
    wpt = np.ascontiguousarray(np.asarray(W_phi, np.float32).T)
    x_np = np.asarray(x, np.float32)
    in_maps = [
        dict(x_full=x_np, x_own=x_own[c], w_theta_t=wtt, w_phi_t=wpt,
             idxw=np.ascontiguousarray(idxw[c]) if plan["W_total"] > 0
             else np.zeros((128, 16), np.int16))
        for c in range(cfg.n_cores)
    ]
    runner.prepare(in_maps)
    outs = runner.run()
    t_ns = runner.time_ns(iters=time_iters) if time_iters else None
    res = runner.results(outs)
    out_full = np.empty((cfg.N, CH), np.float32)
    for c in range(cfg.n_cores):
        out_full[own_nodes[c]] = res[c]["out"][: cfg.npc]
    out_full[deg == 0] = 0.0
    return out_full, t_ns


def kernel(x, edge_index, W_theta, W_phi):
    out, _ = run_cfg(x, edge_index, W_theta, W_phi, Cfg())
    return out



# revision 2
# speedup vs baseline: 3.3396x; 3.3396x over previous
"""Trainium2 Bass kernel for DevConv-style GNN message passing.

Reference computation:
    rel_t = (x[row] - x[col]) @ W_theta.T          # [E, 128]
    aggr  = segment_max(rel_t, row, N)             # [N, 128], empty -> 0
    out   = aggr @ W_phi.T                         # [N, 128]

Key reformulation: with y = x @ W_theta.T, within a segment (fixed row d)
    max_e (y[d] - y[col_e]) = y[d] - min_e y[col_e]     (per channel)
so the per-edge matmul disappears and only ONE gather per edge (y[col]) is
needed, followed by a segmented min.

Distribution: nodes are assigned to the 8 cores by degree-rank striping
(rank r -> core r % 8). Each core:
  Phase A: computes y = x @ W_theta.T for ALL nodes (bf16) into an HBM
           table that is split into 4 chunks of <=32767 rows (+1 sentinel
           row of +3e38 per chunk) because dma_gather indices are int16.
  Phase B: for each 128-node tile and each chunk, gathers y[col] rows into
           a padded [128 nodes x B slots] SBUF rect via dma_gather
           (pad slots point at the chunk sentinel), then pairwise-min folds
           the slots and merges chunks -> m[d] = min_e y[col_e].
  Phase C: aggr = y_own - m  (y_own computed on-chip from x_own),
           out_tile = aggr @ W_phi.T via PE transpose + matmul.
Host un-permutes the concatenated core outputs and zeroes empty nodes.
"""
import sys
import os

sys.path.insert(0, "/opt/trn_rl_repo")

from contextlib import ExitStack
from dataclasses import dataclass

import numpy as np
import ml_dtypes

import concourse.bass as bass
import concourse.tile as tile
from concourse import bacc, mybir
from concourse.masks import make_identity

import time

import jax
from jax.sharding import Mesh, PartitionSpec
from jax.experimental.shard_map import shard_map

from concourse.bass2jax import (
    _bass_exec_p, install_neuronx_cc_hook, partition_id_tensor)


class BassRunner:
    """Keeps a jitted PJRT executable for a Bass program so it can be run
    repeatedly on device-resident inputs (for wall-clock timing)."""

    def __init__(self, nc, n_cores: int):
        install_neuronx_cc_hook()
        self.nc = nc
        self.n_cores = n_cores
        partition_name = nc.partition_id_tensor.name if nc.partition_id_tensor else None
        in_names, out_names, out_avals = [], [], []
        for alloc in nc.m.functions[0].allocations:
            if not isinstance(alloc, mybir.MemoryLocationSet):
                continue
            name = alloc.memorylocations[0].name
            if alloc.kind == "ExternalInput":
                if name != partition_name:
                    in_names.append(name)
            elif alloc.kind == "ExternalOutput":
                out_names.append(name)
                out_avals.append(jax.core.ShapedArray(
                    tuple(alloc.tensor_shape), mybir.dt.np(alloc.dtype)))
        self.in_names, self.out_names, self.out_avals = in_names, out_names, out_avals
        self.n_params = len(in_names)
        all_in_names = list(in_names) + list(out_names)
        if partition_name is not None:
            all_in_names.append(partition_name)

        def _body(*args):
            operands = list(args)
            if partition_name is not None:
                operands.append(partition_id_tensor())
            outs = _bass_exec_p.bind(
                *operands,
                out_avals=tuple(out_avals),
                in_names=tuple(all_in_names),
                out_names=tuple(out_names),
                lowering_input_output_aliases=(),
                sim_require_finite=True,
                sim_require_nnan=True,
                nc=nc,
            )
            return tuple(outs)

        devices = jax.devices()[:n_cores]
        self.mesh = Mesh(np.asarray(devices), ("core",))
        n_outs = len(out_names)
        in_specs = (PartitionSpec("core"),) * (self.n_params + n_outs)
        out_specs = (PartitionSpec("core"),) * n_outs
        self.fn = jax.jit(
            shard_map(_body, mesh=self.mesh, in_specs=in_specs,
                      out_specs=out_specs, check_rep=False),
            keep_unused=True,
        )
        self._dev_args = None

    def prepare(self, in_maps):
        assert len(in_maps) == self.n_cores
        concat_in = [
            np.concatenate([np.asarray(in_maps[c][n]) for c in range(self.n_cores)],
                           axis=0)
            for n in self.in_names
        ]
        concat_zeros = [
            np.zeros((self.n_cores * a.shape[0], *a.shape[1:]), a.dtype)
            for a in self.out_avals
        ]
        sharding = jax.sharding.NamedSharding(self.mesh, PartitionSpec("core"))
        self._dev_args = [jax.device_put(v, sharding) for v in concat_in + concat_zeros]
        return self

    def run(self):
        outs = self.fn(*self._dev_args)
        jax.block_until_ready(outs)
        return outs

    def results(self, outs):
        return [
            {n: np.asarray(outs[i]).reshape(self.n_cores, *self.out_avals[i].shape)[c]
             for i, n in enumerate(self.out_names)}
            for c in range(self.n_cores)
        ]

    def time_ns(self, iters=5, warmup=2):
        for _ in range(warmup):
            self.run()
        ts = []
        for _ in range(iters):
            t0 = time.perf_counter()
            self.run()
            ts.append((time.perf_counter() - t0) * 1e9)
        return min(ts)


CH = 128
F32 = mybir.dt.float32
BF16 = mybir.dt.bfloat16
I16 = mybir.dt.int16
SENT_VAL = 3.0e38
IDX_PER_INST = 1024          # dma_gather crashes >= 2048 idx/instruction
BLK_PER_INST = IDX_PER_INST // 128


@dataclass(frozen=True)
class Cfg:
    N: int = 100_000
    E: int = 3_200_000
    n_cores: int = 8
    chunk_real: int = 25_088     # table rows per chunk (512-aligned, < int16 max)
    idx_per_inst: int = 1024
    balanced: bool = True        # balanced chunk coloring + count-vector tiling

    @property
    def n_chunks(self):
        return (self.N + self.chunk_real - 1) // self.chunk_real

    @property
    def chunk_stride(self):
        return self.chunk_real + 1

    @property
    def npc(self):
        assert self.N % self.n_cores == 0
        return self.N // self.n_cores

    @property
    def tiles(self):
        return (self.npc + 127) // 128


def _wrap16(seg: np.ndarray) -> np.ndarray:
    """Per-instruction idx wrap: flat [n] -> [128, n//16]; idx i at
    (partition i%16, col i//16), replicated across the 8 gpsimd groups."""
    n = seg.shape[-1]
    w = seg.reshape(*seg.shape[:-1], n // 16, 16)
    w = np.swapaxes(w, -1, -2)                       # [..., 16, n//16]
    return np.tile(w, (1,) * (seg.ndim - 1) + (8, 1))


def _color_chunks(row, col, deg, N, NK, cap):
    """Greedy quota-balanced assignment of cols to NK chunks; returns
    (chunk_of, cnt_dk) where cnt_dk[d,k] = per-dest per-chunk edge count."""
    o = np.argsort(col, kind="stable")
    dest_s = row[o]
    starts = np.searchsorted(col[o], np.arange(N + 1))
    quota = -(-deg // NK)
    cnt_dk = np.zeros((N, NK), np.int32)
    chunk_of = np.full(N, -1, np.int64)
    chunk_sz = np.zeros(NK, np.int64)
    col_order = np.argsort(-(starts[1:] - starts[:-1]), kind="stable")
    for _ in range(2):
        for c in col_order:
            ds = dest_s[starts[c]: starts[c + 1]]
            kprev = chunk_of[c]
            if kprev >= 0:
                if len(ds):
                    cnt_dk[ds, kprev] -= 1
                chunk_sz[kprev] -= 1
            if len(ds) == 0:
                k = int(np.argmin(chunk_sz))
            else:
                cc = cnt_dk[ds]
                over = np.maximum(0, cc + 1 - quota[ds][:, None])
                sc = (over * 1000.0 + cc).sum(axis=0).astype(np.float64)
                sc += chunk_sz * 1e-4
                sc[chunk_sz >= cap] = 1e18
                k = int(np.argmin(sc))
                cnt_dk[ds, k] += 1
            chunk_of[c] = k
            chunk_sz[k] += 1
    return chunk_of, cnt_dk


def prep(x, edge_index, cfg: Cfg):
    """Host-side data prep. Returns (plan, per-core inputs, unpermute info)."""
    N, E, NC = cfg.N, cfg.E, cfg.n_cores
    CR, NK, T = cfg.chunk_real, cfg.n_chunks, cfg.tiles
    row = np.asarray(edge_index[0], dtype=np.int64)
    col = np.asarray(edge_index[1], dtype=np.int64)

    deg = np.bincount(row, minlength=N)
    x_np0 = np.asarray(x, dtype=np.float32)
    if cfg.balanced:
        chunk_of, cnt_dk = _color_chunks(row, col, deg, N, NK, CR)
        # rank within chunk
        oc = np.argsort(chunk_of, kind="stable")
        rank_of = np.empty(N, np.int64)
        csz = np.bincount(chunk_of, minlength=NK)
        cstart = np.concatenate([[0], np.cumsum(csz)])
        rank_of[oc] = np.arange(N) - cstart[chunk_of[oc]]
        # dest order: group similar per-chunk count vectors into tiles
        order = np.arange(N)
        for k in range(NK):
            order = order[np.argsort(-cnt_dk[order, k], kind="stable")]
        # x permuted into table layout [NK*CR, CH]
        x_perm = np.zeros((NK * CR, x_np0.shape[1]), np.float32)
        x_perm[chunk_of * CR + rank_of] = x_np0
    else:
        order = np.argsort(-deg, kind="stable")      # node ids by desc degree
        x_perm = x_np0
    core_of = np.empty(N, np.int64)
    pos_of = np.empty(N, np.int64)
    r = np.arange(N)
    core_of[order] = r % NC
    pos_of[order] = r // NC

    ec = core_of[row]
    ep = pos_of[row]
    if cfg.balanced:
        ek = chunk_of[col]
        elocal = rank_of[col].astype(np.int16)
    else:
        ek = col // CR
        elocal = (col - ek * CR).astype(np.int16)
    et = ep // 128
    ed = ep % 128

    # per-(core,tile,chunk,node) counts and within-group slot index j
    key = ((ec * T + et) * NK + ek) * 128 + ed
    o = np.argsort(key, kind="stable")
    ks = key[o]
    first = np.r_[True, ks[1:] != ks[:-1]]
    run_id = np.cumsum(first) - 1
    run_start = np.flatnonzero(first)
    j = np.arange(E) - run_start[run_id]

    cnt = np.bincount(key, minlength=NC * T * NK * 128).reshape(NC, T, NK, 128)
    B = cnt.max(axis=(0, 3)).astype(np.int64)        # [T, NK] shared structure

    Bf = B.reshape(-1)
    off = np.concatenate([[0], np.cumsum(Bf * 128)])  # slot offset per (t,k)
    total_slots = int(off[-1])

    idx_all = np.full((NC, total_slots), CR, np.int16)   # sentinel local idx
    tk = et[o] * NK + ek[o]
    pos_in = off[tk] + j * 128 + ed[o]
    idx_all[ec[o], pos_in] = elocal[o]

    # split into gather instructions and build wrapped idx input
    blk_per_inst = cfg.idx_per_inst // 128
    insts = []           # (t, k, g0blk, nblk, col_off)
    tile_cols = []       # per tile: (col_start, col_end)
    wsegs = []
    col_off = 0
    for t in range(T):
        t_start = col_off
        for k in range(NK):
            btk = int(B[t, k])
            base = int(off[t * NK + k])
            for g0 in range(0, btk, blk_per_inst):
                nb = min(blk_per_inst, btk - g0)
                n_i = nb * 128
                seg = idx_all[:, base + g0 * 128: base + g0 * 128 + n_i]
                wsegs.append(_wrap16(seg))
                insts.append((t, k, g0, nb, col_off))
                col_off += n_i // 16
        tile_cols.append((t_start, col_off))
    idxw = np.concatenate(wsegs, axis=2) if wsegs else np.zeros((NC, 128, 0), np.int16)
    W_total = idxw.shape[2]

    # per-core x_own in pos order, padded to T*128 rows
    own_nodes = np.empty((NC, cfg.npc), np.int64)
    own_nodes[core_of[order], pos_of[order]] = order  # own_nodes[c, p] = node id
    x_np = np.asarray(x, dtype=np.float32)
    x_own = np.zeros((NC, T * 128, CH), np.float32)
    x_own[:, : cfg.npc] = x_np[own_nodes]

    plan = dict(cfg=cfg, B=B, insts=insts, tile_cols=tile_cols, W_total=W_total)
    return plan, idxw, x_own, own_nodes, deg, x_perm


def build_program(plan, reps=1, phases="abc", exp=None):
    exp = {**dict(gather="on", folds="on", queue="rot", fold_mode="reduce"),
           **(exp or {})}
    cfg: Cfg = plan["cfg"]
    N, NK, CR, T = cfg.N, cfg.n_chunks, cfg.chunk_real, cfg.tiles
    CS = cfg.chunk_stride
    B, insts, tile_cols, W_total = (
        plan["B"], plan["insts"], plan["tile_cols"], plan["W_total"])

    nc = bacc.Bacc(None, target_bir_lowering=False, num_swdge_queues=4)
    NA = NK * CR if cfg.balanced else N      # phase-A row count (table layout)
    x_full = nc.declare_dram_parameter("x_full", [NA, CH], F32, isOutput=False)
    x_own = nc.declare_dram_parameter("x_own", [T * 128, CH], F32, isOutput=False)
    wth = nc.declare_dram_parameter("w_theta_t", [CH, CH], F32, isOutput=False)
    wph = nc.declare_dram_parameter("w_phi_t", [CH, CH], F32, isOutput=False)
    idxw = nc.declare_dram_parameter("idxw", [128, max(W_total, 16)], I16, isOutput=False)
    out = nc.declare_dram_parameter("out", [T * 128, CH], F32, isOutput=True)

    qc = [0]  # gather queue rotation

    with tile.TileContext(nc) as tc:
        with ExitStack() as ctx:
            consts = ctx.enter_context(tc.tile_pool(name="consts", bufs=1))
            dram = ctx.enter_context(tc.tile_pool(name="dram", bufs=1, space="DRAM"))
            ax = ctx.enter_context(tc.tile_pool(name="ax", bufs=2))
            axT = ctx.enter_context(tc.tile_pool(name="axT", bufs=2))
            ay = ctx.enter_context(tc.tile_pool(name="ay", bufs=2))
            ps_t = ctx.enter_context(tc.tile_pool(name="ps_t", bufs=2, space="PSUM"))
            ps_y = ctx.enter_context(tc.tile_pool(name="ps_y", bufs=2, space="PSUM"))
            ps_c = ctx.enter_context(tc.tile_pool(name="ps_c", bufs=2, space="PSUM"))
            gidx = ctx.enter_context(tc.tile_pool(name="gidx", bufs=3))
            gdst = ctx.enter_context(tc.tile_pool(name="gdst", bufs=2))
            fold = ctx.enter_context(tc.tile_pool(name="fold", bufs=2))
            fin = ctx.enter_context(tc.tile_pool(name="fin", bufs=2))

            y_aug = dram.tile([NK * CS, CH], BF16)

            ident = consts.tile([128, 128], F32)
            make_identity(nc, ident[:])
            wth_sb = consts.tile([CH, CH], F32)
            nc.sync.dma_start(out=wth_sb[:], in_=wth[:])
            wph_sb = consts.tile([CH, CH], F32)
            nc.sync.dma_start(out=wph_sb[:], in_=wph[:])
            y_own_sb = consts.tile([128, T * 128], F32)
            probe = consts.tile([128, CH], BF16)
            nc.gpsimd.memset(probe[:], 0.0)
            cst128 = consts.tile([128, CH], BF16)
            nc.gpsimd.memset(cst128[:], 1.0)
            sent = consts.tile([1, CH], BF16)
            nc.gpsimd.memset(sent[:], SENT_VAL)
            for k in range(NK):
                nc.sync.dma_start(out=y_aug[k * CS + CR: k * CS + CR + 1, :], in_=sent[:])

            A_MODE = os.environ.get("A_MODE", "full")
            PS_BUFS = int(os.environ.get("PS_BUFS", "2"))
            # ---------------- Phase A: y_aug = (x @ W_theta.T).bf16 ----------
            def emit_group(src, n0, gn, dst):
                """Process rows [n0, n0+gn) of src -> y into dst.
                dst = ("aug",) writes y_aug rows (with chunk-boundary split),
                dst = ("own",) writes y_own_sb cols."""
                nt = (gn + 127) // 128
                xg = ax.tile([128, nt * 128], F32, tag="xg",
                             bufs=int(os.environ.get("XG_BUFS", "2")))
                xg3 = xg[:].rearrange("p (i c) -> p i c", c=CH)
                load_eng = nc.gpsimd if A_MODE == "dma3" else nc.sync
                if gn % 128 == 0:
                    load_eng.dma_start(
                        out=xg3[:, :nt, :],
                        in_=src[n0: n0 + gn, :].rearrange("(i p) c -> p i c", p=128))
                else:
                    for i in range(nt):
                        rn = min(128, gn - i * 128)
                        nc.sync.dma_start(
                            out=xg3[:rn, i, :],
                            in_=src[n0 + i * 128: n0 + i * 128 + rn, :])
                if A_MODE.startswith("dma") and dst == "aug":
                    # dma : load->store dep, both on sync
                    # dma2: stores only dep-free (loads still emitted)
                    # dma3: load on gpsimd, stores dep on load, on sync
                    # dma5: loads only (no stores)
                    if gn % 128 == 0:
                        for i in range(nt):
                            r0 = n0 + i * 128
                            kb = r0 // CR
                            if A_MODE == "dma5":
                                continue
                            src_ap = (cst128[:] if A_MODE == "dma2"
                                      else xg3[:, i, :CH // 2].bitcast(BF16))
                            nc.sync.dma_start(
                                out=y_aug[r0 + kb: r0 + kb + 128, :], in_=src_ap)
                    return
                pt = ps_t.tile([128, nt * 128], F32, tag="pt", bufs=PS_BUFS)
                for i in range(nt):
                    rn = min(128, gn - i * 128)
                    nc.tensor.transpose(
                        out=pt[:, i * 128: i * 128 + rn],
                        in_=xg3[:rn, i, :],
                        identity=ident[:rn, :rn])
                xT = axT.tile([128, nt * 128], F32, tag="xT")
                if exp.get("fold_mode") == "reduce":
                    nc.scalar.copy(out=xT[:, : nt * 128], in_=pt[:, : nt * 128])
                else:
                    nc.vector.tensor_copy(out=xT[:, : nt * 128], in_=pt[:, : nt * 128])
                if A_MODE == "nomm" and dst == "aug":
                    for i in range(nt):
                        r0 = n0 + i * 128
                        kb = r0 // CR
                        nc.sync.dma_start(
                            out=y_aug[r0 + kb: r0 + kb + 128, :],
                            in_=xT[:, i * 128: i * 128 + 128][:, :CH // 2].bitcast(BF16))
                    return
                py = ps_y.tile([128, nt * 128], F32, tag="py", bufs=PS_BUFS)
                for i in range(nt):
                    rn = min(128, gn - i * 128)
                    nc.tensor.matmul(
                        out=py[:rn, i * 128: (i + 1) * 128],
                        lhsT=xT[:, i * 128: i * 128 + rn],
                        rhs=wth_sb[:],
                        start=True, stop=True)
                if dst == "own":
                    nc.scalar.copy(
                        out=y_own_sb[:, n0: n0 + nt * 128], in_=py[:, : nt * 128])
                    return
                yg = ay.tile([128, nt * 128], BF16, tag="yg")
                copy2 = nc.vector.tensor_copy if A_MODE == "dvecopy" else nc.scalar.copy
                if gn % 128 == 0:
                    copy2(out=yg[:, : gn], in_=py[:, : gn])
                else:
                    for i in range(nt):
                        rn = min(128, gn - i * 128)
                        copy2(
                            out=yg[:rn, i * 128: (i + 1) * 128],
                            in_=py[:rn, i * 128: (i + 1) * 128])
                yg3 = yg[:].rearrange("p (i c) -> p i c", c=CH)
                # write y rows n -> aug rows n + n // CR, splitting at tile level
                for i in range(nt):
                    r0 = n0 + i * 128
                    rn = min(128, gn - i * 128)
                    kb = r0 // CR
                    ke = (r0 + rn - 1) // CR
                    if kb == ke:
                        nc.sync.dma_start(
                            out=y_aug[r0 + kb: r0 + kb + rn, :], in_=yg3[:rn, i, :])
                    else:
                        split = (kb + 1) * CR - r0       # rows before boundary
                        nc.sync.dma_start(
                            out=y_aug[r0 + kb: r0 + kb + split, :],
                            in_=yg3[:split, i, :])
                        nc.sync.dma_start(
                            out=y_aug[r0 + split + ke: r0 + ke + rn, :],
                            in_=yg3[split:rn, i, :])

            by_tile = {}
            for (t, k, g0, nb, coff) in insts:
                by_tile.setdefault(t, []).append((k, g0, nb, coff))

            for _rep in range(reps):
              for n0 in range(0, NA, 512):
                emit_group(x_full, n0, min(512, NA - n0), "aug")
              for n0 in range(0, T * 128, 512):
                emit_group(x_own, n0, min(512, T * 128 - n0), "own")

              # ---------------- Phase B + C per tile ---------------------------
              for t in range(T):
                 c0, c1 = tile_cols[t]
                 it = gidx.tile([128, max(c1 - c0, 16)], I16, tag="it")
                 if c1 > c0:
                     nc.sync.dma_start(out=it[:, : c1 - c0], in_=idxw[:, c0:c1])
                 if exp.get("fold_mode") == "reduce":
                     kws = [k for k in range(NK) if int(B[t, k]) > 0]
                     koff = {}
                     wt = 0
                     for k in kws:
                         koff[k] = wt
                         wt += int(B[t, k])
                     if wt > 0:
                         dk = gdst.tile([128, wt * CH], BF16, tag="gr")
                         dk3 = dk[:].rearrange("p (b c) -> p b c", c=CH)
                     for (k, g0, nb, coff) in by_tile.get(t, []):
                         if exp["gather"] == "off":
                             break
                         n_i = nb * 128
                         col0 = koff[k] + g0
                         nc.gpsimd.dma_gather(
                             out_ap=dk3[:, col0: col0 + nb, :],
                             in_ap=y_aug[k * CS: (k + 1) * CS, :],
                             idxs_ap=it[:, coff - c0: coff - c0 + n_i // 16],
                             num_idxs=n_i,
                             num_idxs_reg=n_i,
                             elem_size=CH,
                             queue_num=(qc[0] % 4) if exp["queue"] == "rot" else 0,
                         )
                         qc[0] += 1
                     m = fin.tile([128, CH], F32, tag="m")
                     if wt == 0:
                         nc.gpsimd.memset(m[:], SENT_VAL)
                     else:
                         dkT = dk[:].rearrange("p (b c) -> p c b", c=CH)
                         nc.vector.tensor_reduce(
                             out=m[:], in_=dkT, axis=mybir.AxisListType.X,
                             op=mybir.AluOpType.min)
                     aggr = fin.tile([128, CH], F32, tag="aggr")
                     nc.vector.tensor_sub(
                         out=aggr[:], in0=y_own_sb[:, t * 128: (t + 1) * 128],
                         in1=m[:])
                     ptr = ps_c.tile([128, CH], F32, tag="ctr")
                     nc.tensor.transpose(out=ptr[:], in_=aggr[:], identity=ident[:])
                     aggrT = fin.tile([128, CH], F32, tag="aggrT")
                     nc.scalar.copy(out=aggrT[:], in_=ptr[:])
                     po = ps_c.tile([128, CH], F32, tag="cmm")
                     nc.tensor.matmul(out=po[:], lhsT=aggrT[:], rhs=wph_sb[:],
                                      start=True, stop=True)
                     osb = fin.tile([128, CH], F32, tag="osb")
                     nc.scalar.copy(out=osb[:], in_=po[:])
                     nc.sync.dma_start(out=out[t * 128: (t + 1) * 128, :],
                                       in_=osb[:])
                     continue
                 dks = {}
                 for k in range(NK):
                     btk = int(B[t, k])
                     if btk == 0:
                         continue
                     dks[k] = gdst.tile([128, btk * CH], BF16, tag=f"g{k}", name=f"dk{k}")
                 for (k, g0, nb, coff) in by_tile.get(t, []):
                     if exp["gather"] == "off":
                         break
                     dk3 = dks[k][:].rearrange("p (b c) -> p b c", c=CH)
                     n_i = nb * 128
                     ndup = 2 if exp["gather"] == "dup" else 1
                     for di in range(ndup):
                         if di == 0:
                             dst = dk3[:, g0: g0 + nb, :]
                         else:
                             ddup = gdst.tile(
                                 [128, (cfg.idx_per_inst // 128) * CH], BF16,
                                 tag="gdup", bufs=2)
                             dst = ddup[:].rearrange(
                                 "p (b c) -> p b c", c=CH)[:, :nb, :]
                         nc.gpsimd.dma_gather(
                             out_ap=dst,
                             in_ap=y_aug[k * CS: (k + 1) * CS, :],
                             idxs_ap=it[:, coff - c0: coff - c0 + n_i // 16],
                             num_idxs=n_i,
                             num_idxs_reg=n_i,
                             elem_size=CH,
                             queue_num=(qc[0] % 4) if exp["queue"] == "rot" else 0,
                         )
                         qc[0] += 1
                 # fold each chunk's rect down to one [128, CH] min
                 mks = []
                 if exp["folds"] == "off":
                     m = fin.tile([128, CH], F32, tag="m")
                     nc.gpsimd.memset(m[:], SENT_VAL)
                     aggr = fin.tile([128, CH], F32, tag="aggr")
                     nc.vector.tensor_sub(
                         out=aggr[:], in0=y_own_sb[:, t * 128: (t + 1) * 128],
                         in1=m[:])
                     ptr = ps_c.tile([128, CH], F32, tag="ctr")
                     nc.tensor.transpose(out=ptr[:], in_=aggr[:], identity=ident[:])
                     aggrT = fin.tile([128, CH], F32, tag="aggrT")
                     nc.vector.tensor_copy(out=aggrT[:], in_=ptr[:])
                     po = ps_c.tile([128, CH], F32, tag="cmm")
                     nc.tensor.matmul(out=po[:], lhsT=aggrT[:], rhs=wph_sb[:],
                                      start=True, stop=True)
                     osb = fin.tile([128, CH], F32, tag="osb")
                     nc.scalar.copy(out=osb[:], in_=po[:])
                     nc.sync.dma_start(out=out[t * 128: (t + 1) * 128, :], in_=osb[:])
                     continue
                 for k in range(NK):
                     if k not in dks:
                         continue
                     cur = dks[k]
                     nb = int(B[t, k])
                     while nb > 1:
                         half = (nb + 1) // 2
                         nxt = fold.tile([128, half * CH], BF16, tag=f"f{k}", bufs=3)
                         nc.vector.tensor_tensor(
                             out=nxt[:, : half * CH],
                             in0=cur[:, : half * CH],
                             in1=cur[:, (nb - half) * CH: nb * CH],
                             op=mybir.AluOpType.min)
                         cur, nb = nxt, half
                     mks.append(cur)
                 m = fin.tile([128, CH], F32, tag="m")
                 if len(mks) == 0:
                     nc.gpsimd.memset(m[:], SENT_VAL)
                 elif len(mks) == 1:
                     nc.vector.tensor_copy(out=m[:], in_=mks[0][:, :CH])
                 else:
                     # sequential accumulate with alternating tags (max 2 live)
                     acc = mks[0]
                     for i in range(1, len(mks) - 1):
                         mm = fold.tile([128, CH], BF16, tag=f"mrg{i % 2}")
                         nc.vector.tensor_tensor(
                             out=mm[:], in0=acc[:, :CH], in1=mks[i][:, :CH],
                             op=mybir.AluOpType.min)
                         acc = mm
                     nc.vector.tensor_tensor(
                         out=m[:], in0=acc[:, :CH], in1=mks[-1][:, :CH],
                         op=mybir.AluOpType.min)
                 # aggr = y_own - m ; out_tile = aggr @ W_phi.T
                 aggr = fin.tile([128, CH], F32, tag="aggr")
                 nc.vector.tensor_sub(
                     out=aggr[:], in0=y_own_sb[:, t * 128: (t + 1) * 128], in1=m[:])
                 ptr = ps_c.tile([128, CH], F32, tag="ctr")
                 nc.tensor.transpose(out=ptr[:], in_=aggr[:], identity=ident[:])
                 aggrT = fin.tile([128, CH], F32, tag="aggrT")
                 nc.vector.tensor_copy(out=aggrT[:], in_=ptr[:])
                 po = ps_c.tile([128, CH], F32, tag="cmm")
                 nc.tensor.matmul(out=po[:], lhsT=aggrT[:], rhs=wph_sb[:],
                                  start=True, stop=True)
                 osb = fin.tile([128, CH], F32, tag="osb")
                 nc.scalar.copy(out=osb[:], in_=po[:])
                 nc.sync.dma_start(out=out[t * 128: (t + 1) * 128, :], in_=osb[:])

            if phases != "abc":
                fillz = consts.tile([128, CH], F32)
                nc.vector.tensor_copy(out=fillz[:], in_=probe[:])
                for t in range(T):
                    nc.sync.dma_start(out=out[t * 128: (t + 1) * 128, :], in_=fillz[:])
    nc.compile()
    return nc


_CACHE = {}


def _get_runner_and_plan(x, edge_index, cfg: Cfg, reps=1, phases="abc", exp=None):
    plan, idxw, x_own, own_nodes, deg, x_perm = prep(x, edge_index, cfg)
    skey = (cfg, reps, phases, tuple(sorted((exp or {}).items())),
            tuple(plan["B"].reshape(-1).tolist()))
    if skey not in _CACHE:
        nc = build_program(plan, reps=reps, phases=phases, exp=exp)
        _CACHE[skey] = BassRunner(nc, cfg.n_cores)
    return _CACHE[skey], plan, idxw, x_own, own_nodes, deg, x_perm


def run_cfg(x, edge_index, W_theta, W_phi, cfg: Cfg, time_iters=0, reps=1, phases="abc", exp=None):
    runner, plan, idxw, x_own, own_nodes, deg, x_perm = _get_runner_and_plan(x, edge_index, cfg, reps=reps, phases=phases, exp=exp)
    if exp and exp.get("sent_idx"):
        idxw = np.full_like(idxw, cfg.chunk_real)
    wtt = np.ascontiguousarray(np.asarray(W_theta, np.float32).T)
    wpt = np.ascontiguousarray(np.asarray(W_phi, np.float32).T)
    in_maps = [
        dict(x_full=x_perm, x_own=x_own[c], w_theta_t=wtt, w_phi_t=wpt,
             idxw=np.ascontiguousarray(idxw[c]) if plan["W_total"] > 0
             else np.zeros((128, 16), np.int16))
        for c in range(cfg.n_cores)
    ]
    runner.prepare(in_maps)
    outs = runner.run()
    t_ns = runner.time_ns(iters=time_iters) if time_iters else None
    res = runner.results(outs)
    out_full = np.empty((cfg.N, CH), np.float32)
    for c in range(cfg.n_cores):
        out_full[own_nodes[c]] = res[c]["out"][: cfg.npc]
    out_full[deg == 0] = 0.0
    return out_full, t_ns


def kernel(x, edge_index, W_theta, W_phi):
    out, _ = run_cfg(x, edge_index, W_theta, W_phi, Cfg())
    return out

